# revision 3
# baseline (speedup 1.0000x reference)
"""Trainium2 Bass kernel v3 for ChipletThermalModel.

Math per chiplet i, grid point g (b/c in chiplet-scaled coords):
  bm = -x/lx + (w2+cx)/lx ; bp = x/lx + (w2-cx)/lx   (same for c via y/ly)
  For the 4 combos (kx,ky):
    lS = ln(a^2 + b^2 + c^2); delta = exp(lS/2); rd = exp(-lS/2 - ln a)
    t1+t2 pair over the opposite sign:
      q_b(kx) = b*(ln((cm+d_kx,m)(cp+d_kx,p)) - ln(a^2+b^2))
    t3: at = Arctan(b*c*rd)
  res += c4*sum(q) + c5*sum(at)   (c4=Pi*A*2/sqrt(pi), c5=-a*c4)
  plus endC = A*B_off*sum(Pi) folded into chiplet 0's update.

All Ln/Exp live in activation-table set 6 (forced via a patched table list
passed to the load-insertion pass), so ACT does 28 ops/chiplet with exactly
2 table loads (set6 + arctan set2; Square is in every set). The atan-arg
path runs in fp16 on DVE (2x mode). Pool does the s0/cpd/bpd adds + one
square + one partial sum. Emission is software-pipelined: chiplet i+1's
head runs before chiplet i's tail; reductions run one iteration later.
"""
import sys
import numpy as np

for _p in ("/opt/trn_rl_repo",):
    if _p not in sys.path:
        sys.path.insert(0, _p)

N_CORES = 8
B, NCHIP, G2 = 64, 16, 65536
RPC = B // N_CORES            # batch rows per core = 8
P = 128                       # SBUF partitions
F = RPC * G2 // P             # free-dim columns per core = 4096
W = 1024                      # columns per processing group
NG = F // W                   # groups
REP = P // RPC                # partitions per batch row = 16
NPAR = 6 * NCHIP + 1          # params columns (6 per chiplet + endC)
C1 = float(2.0 / np.sqrt(np.pi))
KX = ("m", "p")


def _patched_tables(orig_fn, mybir):
    """Return a get_activation_tables wrapper that strips Ln/Exp from every
    set except index 6, so the load-insertion pass must pick set 6 (which
    genuinely holds both ln and exp) instead of thrashing sets 5/0."""
    AF = mybir.ActivationFunctionType

    def patched(arch):
        out = {}
        for idx, (name, s) in enumerate(orig_fn(arch).items()):
            s2 = set(s)
            if idx != 6:
                s2.discard(AF.Ln)
                s2.discard(AF.Exp)
            out[name] = s2
        return out

    return patched


def _build_program(scal):
    from concourse import bacc, tile
    import concourse.mybir as mybir

    AF = mybir.ActivationFunctionType
    OP = mybir.AluOpType
    FP32 = mybir.dt.float32
    FP16 = mybir.dt.float16

    a2 = scal["a2"]
    nla = scal["neg_ln_a"]
    inv_lx = scal["inv_lx"]
    inv_ly = scal["inv_ly"]

    nc = bacc.Bacc("TRN2", target_bir_lowering=False, debug=False,
                   enable_asserts=False)

    xin = nc.dram_tensor("xin", [P, F], FP32, kind="ExternalInput")
    yin = nc.dram_tensor("yin", [P, F], FP32, kind="ExternalInput")
    prm = nc.dram_tensor("prm", [P, NPAR], FP32, kind="ExternalInput")
    out = nc.dram_tensor("out", [P, F], FP32, kind="ExternalOutput")

    with tile.TileContext(nc) as tc:
        with tc.tile_pool(name="cst", bufs=1) as cst, \
             tc.tile_pool(name="io", bufs=2) as io, \
             tc.tile_pool(name="wk", bufs=1) as wk:
            prmt = cst.tile([P, NPAR], FP32)
            nc.sync.dma_start(prmt[:], prm[:])

            def pcol(i, k):           # [128,1] per-partition param AP
                return prmt[:, 6 * i + k: 6 * i + k + 1]

            endC = prmt[:, 6 * NCHIP: 6 * NCHIP + 1]

            for g in range(NG):
                cs = slice(g * W, (g + 1) * W)
                xt = io.tile([P, W], FP32, tag="xt")
                yt = io.tile([P, W], FP32, tag="yt")
                res = io.tile([P, W], FP32, tag="res")
                nc.sync.dma_start(xt[:], xin[:, cs])
                nc.sync.dma_start(yt[:], yin[:, cs])

                def head(i):
                    """b/c (DVE TS), squares (2 ACT + 1 DVE + 1 Pool),
                    s0 = b^2+c^2 (Pool) for chiplet i."""
                    st = {"i": i, "bs": {}, "cs": {}, "sqb": {}, "sqc": {},
                          "s0": {}}
                    for k, sgn, col in (("m", -1.0, 0), ("p", 1.0, 1)):
                        b = wk.tile([P, W], FP32, tag="bc", bufs=8, name="b")
                        nc.vector.tensor_scalar(b[:], xt[:], sgn * inv_lx[i],
                                                pcol(i, col), OP.mult, OP.add)
                        st["bs"][k] = b
                    for k, sgn, col in (("m", -1.0, 2), ("p", 1.0, 3)):
                        c = wk.tile([P, W], FP32, tag="bc", bufs=8, name="c")
                        nc.vector.tensor_scalar(c[:], yt[:], sgn * inv_ly[i],
                                                pcol(i, col), OP.mult, OP.add)
                        st["cs"][k] = c
                    for k in KX:
                        sb = wk.tile([P, W], FP32, tag="sq", bufs=8, name="sb")
                        nc.scalar.activation(sb[:], st["bs"][k][:], AF.Square)
                        st["sqb"][k] = sb
                    scm = wk.tile([P, W], FP32, tag="sq", bufs=8, name="scm")
                    nc.vector.tensor_tensor(scm[:], st["cs"]["m"][:],
                                            st["cs"]["m"][:], OP.mult)
                    st["sqc"]["m"] = scm
                    scp = wk.tile([P, W], FP32, tag="sq", bufs=8, name="scp")
                    nc.gpsimd.tensor_tensor(scp[:], st["cs"]["p"][:],
                                            st["cs"]["p"][:], OP.mult)
                    st["sqc"]["p"] = scp
                    for kx in KX:
                        for ky in KX:
                            s0 = wk.tile([P, W], FP32, tag="s0", bufs=8,
                                         name="s0")
                            nc.gpsimd.tensor_tensor(
                                s0[:], st["sqb"][kx][:], st["sqc"][ky][:],
                                OP.add)
                            st["s0"][kx + ky] = s0
                    return st

                def mid(st):
                    """ACT set6 block (lS, rd, delta, lax), bc (DVE fp16),
                    cpd/bpd (Pool), targ (DVE fp16)."""
                    # Per combo: lS = Ln(s0+a^2) (in place), rd = exp(-lS/2
                    # - ln a) = 1/(a*delta) in fp16, delta = exp(lS/2) (in
                    # place over lS after rd read it). Interleaved per combo
                    # so delta_mm is ready after 3 ACT ops and Pool's cpd
                    # adds start early.
                    rds = {}
                    for kk in ("mm", "mp", "pm", "pp"):
                        nc.scalar.activation(st["s0"][kk][:], st["s0"][kk][:],
                                             AF.Ln, bias=a2)
                        rd = wk.tile([P, W], FP16, tag="rd", bufs=6,
                                     name="rd")
                        nc.scalar.activation(rd[:], st["s0"][kk][:], AF.Exp,
                                             scale=-0.5, bias=nla)
                        rds[kk] = rd
                        nc.scalar.activation(st["s0"][kk][:], st["s0"][kk][:],
                                             AF.Exp, scale=0.5)
                    st["rd"] = rds
                    st["dl"] = st["s0"]
                    # lax = Ln(sq + a^2), in place
                    for k in KX:
                        nc.scalar.activation(st["sqb"][k][:],
                                             st["sqb"][k][:], AF.Ln, bias=a2)
                        nc.scalar.activation(st["sqc"][k][:],
                                             st["sqc"][k][:], AF.Ln, bias=a2)
                    st["laxb"], st["laxc"] = st["sqb"], st["sqc"]
                    # bc = b*c (DVE, fp16 out)
                    bcs = {}
                    for kx in KX:
                        for ky in KX:
                            bc = wk.tile([P, W], FP16, tag="bct", bufs=5,
                                         name="bc")
                            nc.vector.tensor_tensor(bc[:], st["bs"][kx][:],
                                                    st["cs"][ky][:], OP.mult)
                            bcs[kx + ky] = bc
                    st["bc"] = bcs
                    # c+d / b+d (Pool), prod-pair-enable order
                    cpds = {}
                    bpds = {}

                    def _add(dst_src, kk, nm):
                        tl = wk.tile([P, W], FP32, tag="cpd", bufs=12,
                                     name=nm)
                        nc.gpsimd.tensor_tensor(
                            tl[:], dst_src[:], st["dl"][kk][:], OP.add)
                        return tl

                    cpds["mm"] = _add(st["cs"]["m"], "mm", "cpd")
                    cpds["mp"] = _add(st["cs"]["p"], "mp", "cpd")
                    bpds["mm"] = _add(st["bs"]["m"], "mm", "bpd")
                    bpds["pm"] = _add(st["bs"]["p"], "pm", "bpd")
                    cpds["pm"] = _add(st["cs"]["m"], "pm", "cpd")
                    cpds["pp"] = _add(st["cs"]["p"], "pp", "cpd")
                    bpds["mp"] = _add(st["bs"]["m"], "mp", "bpd")
                    bpds["pp"] = _add(st["bs"]["p"], "pp", "bpd")
                    st["cpd"], st["bpd"] = cpds, bpds
                    # targ = bc*rd (DVE fp16 2x, in place over bc)
                    for kk in ("mm", "mp", "pm", "pp"):
                        nc.vector.tensor_tensor(bcs[kk][:], bcs[kk][:],
                                                rds[kk][:], OP.mult)

                def tail(st):
                    """prod/lnp/Lb/q chain + atan activations."""
                    bs, cs_ = st["bs"], st["cs"]
                    cpds, bpds = st["cpd"], st["bpd"]
                    prods = [cpds["mm"], bpds["mm"], cpds["pm"], bpds["mp"]]
                    laxs = [st["laxb"]["m"], st["laxc"]["m"],
                            st["laxb"]["p"], st["laxc"]["p"]]
                    wts = [bs["m"], cs_["m"], bs["p"], cs_["p"]]
                    nc.vector.tensor_tensor(prods[0][:], prods[0][:],
                                            cpds["mp"][:], OP.mult)
                    nc.vector.tensor_tensor(prods[1][:], prods[1][:],
                                            bpds["pm"][:], OP.mult)
                    nc.vector.tensor_tensor(prods[2][:], prods[2][:],
                                            cpds["pp"][:], OP.mult)
                    nc.vector.tensor_tensor(prods[3][:], prods[3][:],
                                            bpds["pp"][:], OP.mult)
                    # lnp (ACT set6, in place)
                    for pr in prods:
                        nc.scalar.activation(pr[:], pr[:], AF.Ln)
                    # atan (ACT set2, fp16 out); arg = b*c/(a*delta) via rd
                    ats = []
                    for kk in ("mm", "mp", "pm", "pp"):
                        at = wk.tile([P, W], FP16, tag="at", bufs=8,
                                     name="at")
                        nc.scalar.activation(at[:], st["bc"][kk][:],
                                             AF.Arctan)
                        ats.append(at)
                    st["ats"] = ats
                    # Lb = lnp - lax ; q = w*Lb (DVE, in place)
                    qs = []
                    for pr, lax, w_ in zip(prods, laxs, wts):
                        nc.vector.tensor_tensor(pr[:], pr[:], lax[:],
                                                OP.subtract)
                        nc.vector.tensor_tensor(pr[:], w_[:], pr[:], OP.mult)
                        qs.append(pr)
                    st["qs"] = qs

                def redtail(st):
                    """Sums + res updates for a previous chiplet."""
                    i = st["i"]
                    ats, qs = st["ats"], st["qs"]
                    nc.vector.tensor_tensor(ats[0][:], ats[0][:], ats[1][:],
                                            OP.add)
                    nc.vector.tensor_tensor(ats[2][:], ats[2][:], ats[3][:],
                                            OP.add)
                    nc.vector.tensor_tensor(ats[0][:], ats[0][:], ats[2][:],
                                            OP.add)
                    nc.vector.tensor_tensor(qs[0][:], qs[0][:], qs[1][:],
                                            OP.add)
                    nc.gpsimd.tensor_tensor(qs[2][:], qs[2][:], qs[3][:],
                                            OP.add)
                    nc.vector.tensor_tensor(qs[0][:], qs[0][:], qs[2][:],
                                            OP.add)
                    if i == 0:
                        nc.vector.tensor_scalar(res[:], qs[0][:], pcol(i, 4),
                                                endC, OP.mult, OP.add)
                    else:
                        nc.vector.scalar_tensor_tensor(res[:], qs[0][:],
                                                       pcol(i, 4), res[:],
                                                       OP.mult, OP.add)
                    nc.vector.scalar_tensor_tensor(res[:], ats[0][:],
                                                   pcol(i, 5), res[:],
                                                   OP.mult, OP.add)

                st = head(0)
                prev = None
                for i in range(NCHIP):
                    mid(st)
                    nxt = head(i + 1) if i + 1 < NCHIP else None
                    tail(st)
                    if prev is not None:
                        redtail(prev)
                    prev = st
                    st = nxt
                redtail(prev)
                nc.sync.dma_start(out[:, cs], res[:])

    import concourse.bacc as bacc_mod
    orig_fn = bacc_mod.get_activation_tables
    bacc_mod.get_activation_tables = _patched_tables(orig_fn, mybir)
    try:
        nc.finalize()
    finally:
        bacc_mod.get_activation_tables = orig_fn
    return nc


def _host_params(cx, cy, w, h, Pw, A, a, B_off, lx, ly, rows):
    """Per-core [128, NPAR] parameter matrix (per-partition scalars)."""
    pr = np.zeros((P, NPAR), dtype=np.float32)
    for i in range(NCHIP):
        w2 = 0.5 * w[rows, i]
        h2 = 0.5 * h[rows, i]
        c4 = Pw[rows, i] * A * C1
        pr[:, 6 * i + 0] = np.repeat((w2 + cx[rows, i]) / lx[i], REP)
        pr[:, 6 * i + 1] = np.repeat((w2 - cx[rows, i]) / lx[i], REP)
        pr[:, 6 * i + 2] = np.repeat((h2 + cy[rows, i]) / ly[i], REP)
        pr[:, 6 * i + 3] = np.repeat((h2 - cy[rows, i]) / ly[i], REP)
        pr[:, 6 * i + 4] = np.repeat(c4, REP)
        pr[:, 6 * i + 5] = np.repeat(-a * c4, REP)
    pr[:, 6 * NCHIP] = np.repeat(A * B_off * Pw[rows].sum(axis=1), REP)
    return np.ascontiguousarray(pr, dtype=np.float32)


_CACHE = {}


def run(x, y, chiplets_x, chiplets_y, chiplets_width, chiplets_height,
        chiplets_power, A, a, B_off, lx, ly, grid=None, trace=False):
    from concourse import bass_utils

    x = np.asarray(x, dtype=np.float32)
    y = np.asarray(y, dtype=np.float32)
    cx = np.asarray(chiplets_x, dtype=np.float32)
    cy = np.asarray(chiplets_y, dtype=np.float32)
    w = np.asarray(chiplets_width, dtype=np.float32)
    h = np.asarray(chiplets_height, dtype=np.float32)
    Pw = np.asarray(chiplets_power, dtype=np.float32)
    Af = float(np.asarray(A).reshape(-1)[0])
    af = float(np.asarray(a).reshape(-1)[0])
    Bf = float(np.asarray(B_off).reshape(-1)[0])
    lxf = np.asarray(lx, dtype=np.float64)
    lyf = np.asarray(ly, dtype=np.float64)

    scal = {
        "a2": float(af * af),
        "neg_ln_a": float(-np.log(af)),
        "inv_lx": [float(1.0 / lxf[i]) for i in range(NCHIP)],
        "inv_ly": [float(1.0 / lyf[i]) for i in range(NCHIP)],
    }
    if "nc" not in _CACHE:
        _CACHE["nc"] = _build_program(scal)
    nc = _CACHE["nc"]

    in_maps = []
    for c in range(N_CORES):
        rows = slice(c * RPC, (c + 1) * RPC)
        xs = np.ascontiguousarray(x[rows].reshape(P, F))
        ys = np.ascontiguousarray(y[rows].reshape(P, F))
        pr = _host_params(cx, cy, w, h, Pw, Af, af, Bf, lxf, lyf, rows)
        in_maps.append({"xin": xs, "yin": ys, "prm": pr})

    rr = bass_utils.run_bass_kernel_spmd(
        nc, in_maps, core_ids=list(range(N_CORES)), trace=trace)

    outs = []
    for c in range(N_CORES):
        o = np.asarray(rr.results[c]["out"], dtype=np.float32)
        outs.append(o.reshape(RPC, G2))
    full = np.concatenate(outs, axis=0)
    if trace:
        return full, rr
    return full


def kernel(**inputs):
    return run(**inputs)
